# revision 1
# baseline (speedup 1.0000x reference)
"""Trainium2 Bass kernel for the SLAYER-style 2-layer spiking MLP.

Reference computation (per batch element n):
    flat   = input.reshape(64, 3072)
    a1     = flat @ w1.T                      (constant over time)
    u1[t]  = a1 * c[t]          where c = cumsum(srm kernel)  (PSP of a
             time-constant input is just a ramp scale)
    s1     = spike_scan(u1)     sequential threshold w/ refractory feedback
    a2[t]  = w2 @ s1[:, t]
    u2     = psp(a2)            (true temporal conv, srm kernel)
    out    = spike_scan(u2)

Key algebraic facts exploited on-device:
  * The refractory kernel rk[d] = -20*d*e^{1-d} (d=1..32) is
    polynomial-geometric, so the refractory sum r[t] = sum_d rk[d] s[t-d]
    follows an exact order-2 recurrence:
        P[t] = q*P[t-1] + s[t-1]
        R[t] = q*R[t-1] + P[t]          (q = e^-1, states scaled by -1/20)
        spike:  u + (-20)*R >= theta  <=>  R + 0.5 <= u/20
    The d>32 truncation of rk is ~1e-11 and far below fp32 noise.
  * Each scan step is exactly 3 fused scalar_tensor_tensor DVE ops over a
    [104, 33] tile holding both layers (layer 2 rides along lagged 32 steps).
  * The SRM PSP filter srm[k] = (k/10)e^{1-k/10} is the same confluent
    geometric form, handled by two hardware tensor_tensor_scan ops per
    16-step block (never truncated within T=100, so it is exact).

Sharding: data-parallel over batch, 8 elements per core, weights replicated.
"""

import numpy as np

NB = 8            # batch elements per core
T = 100           # timesteps
B = 16            # pipeline block size
LAG = 32          # layer-2 ride-along lag (>= refractory window 32)
TF = T + LAG      # fused scan steps
NCOL = 33         # 32 layer-1 columns (4 chunks x 8 batch) + 1 layer-2 column
PMAX = 104        # padded partition count per o-chunk
MC = [103, 103, 102, 102]      # o-chunk sizes (sum = 410)
OFF = [0, 103, 206, 308]
KT = 24           # 3072 / 128 k-tiles
NO1 = 410
NO2 = 10

_CACHE = {}


def _consts():
    q = float(np.float32(np.exp(-1.0)))          # refractory ratio
    p = float(np.float32(np.exp(-0.1)))          # SRM ratio
    k2 = float(np.float32(np.exp(1.0) / 200.0))  # a2 pre-scale: u2/20 = sum
    return q, p, k2


def build():
    import concourse.bass as bass
    import concourse.bacc as bacc
    import concourse.mybir as mybir
    from concourse import tile

    f32 = mybir.dt.float32
    Alu = mybir.AluOpType
    q, p, k2 = _consts()

    nc = bacc.Bacc("TRN2", target_bir_lowering=False, debug=False, num_devices=8)

    flatT_d = nc.dram_tensor("flatT", [KT * 128, NB], f32, kind="ExternalInput")
    w1T_d = nc.dram_tensor("w1T", [KT * 128, NO1], f32, kind="ExternalInput")
    w2p_d = nc.dram_tensor("w2p", [PMAX, 4, NO2], f32, kind="ExternalInput")
    c20_d = nc.dram_tensor("c20rep", [PMAX, T], f32, kind="ExternalInput")
    pc_d = nc.dram_tensor("pconst", [80, B], f32, kind="ExternalInput")
    sel_d = nc.dram_tensor("sel", [NO2, NB, 80], f32, kind="ExternalInput")
    eye_d = nc.dram_tensor("eye8", [NB, NB], f32, kind="ExternalInput")
    out_d = nc.dram_tensor("out", [80, T], f32, kind="ExternalOutput")

    with tile.TileContext(nc) as tc:
        with (
            tc.tile_pool(name="pers", bufs=1) as pool,
            tc.tile_pool(name="ps1", bufs=1, space="PSUM") as ps1,
            tc.tile_pool(name="ps2", bufs=2, space="PSUM") as ps2,
        ):
            w1sb = pool.tile([128, KT, NO1], f32, tag="w1sb")
            fTsb = pool.tile([128, KT, NB], f32, tag="fTsb")
            w2sb = pool.tile([PMAX, 4, NO2], f32, tag="w2sb")
            c20sb = pool.tile([PMAX, T], f32, tag="c20sb")
            pcsb = pool.tile([80, B], f32, tag="pcsb")
            A1 = pool.tile([PMAX, 32], f32, tag="A1")
            Up = pool.tile([PMAX, TF, NCOL], f32, tag="Up")
            # mega-tile: spike history S (TF+1 slots of NCOL) followed by the
            # interleaved IIR state [P(NCOL) | R(NCOL)] — one address space so
            # a single dual-range AP can feed (s_tau | P) to the merged
            # state-update op.
            SW = (TF + 1) * NCOL
            M = pool.tile([PMAX, SW + 2 * NCOL], f32, tag="M")
            selsb = pool.tile([NO2, NB, 80], f32, tag="selsb")
            eyesb = pool.tile([NB, NB], f32, tag="eyesb")
            a1rsb = pool.tile([NB, NO1], f32, tag="a1rsb")
            a2tmp = pool.tile([NO2, B, NB], f32, tag="a2tmp")
            a2s = pool.tile([80, T + 1], f32, tag="a2s")
            W1 = pool.tile([80, T + 1], f32, tag="W1")
            W2 = pool.tile([80, T + 1], f32, tag="W2")
            ostage = pool.tile([80, T], f32, tag="ostage")

            # ---- input DMAs (small tensors first: the stationary matmul
            # operand and constants gate everything else) ----
            nc.sync.dma_start(
                fTsb[:], flatT_d[:].rearrange("(k p) n -> p k n", p=128)
            )
            nc.sync.dma_start(w2sb[:], w2p_d[:])
            nc.sync.dma_start(c20sb[:], c20_d[:])
            nc.sync.dma_start(pcsb[:], pc_d[:])
            nc.sync.dma_start(selsb[:], sel_d[:])
            nc.sync.dma_start(eyesb[:], eye_d[:])
            for k in range(KT):
                for h in range(2):
                    r0, r1 = 64 * h, 64 * (h + 1)
                    nc.sync.dma_start(
                        w1sb[r0:r1, k, :],
                        w1T_d[k * 128 + r0:k * 128 + r1, :],
                    )

            # ---- state init ----
            nc.gpsimd.memset(Up[:], 0.0)
            nc.vector.memset(A1[:], 0.0)
            nc.vector.memset(M[:, 0:NCOL], 0.0)            # S slot 0
            nc.vector.memset(M[:, SW:SW + 2 * NCOL], 0.0)  # P | R
            nc.vector.memset(a2s[:, 0:1], 0.0)
            nc.vector.memset(W1[:, 0:1], 0.0)
            nc.vector.memset(W2[:, 0:1], 0.0)

            # ---- fc1: A1row[n, o] = flat @ w1.T, accumulated over k.
            # flatT tiles are already [K, 8] so they serve as the (tiny)
            # stationary operand; w1T tiles stream as the moving operand.
            a1row = ps1.tile([NB, NO1], f32, tag="a1row", name="a1row")
            for k in range(KT):
                nc.tensor.matmul(
                    a1row[:],
                    fTsb[:, k, :],
                    w1sb[:, k, :],
                    start=(k == 0),
                    stop=(k == KT - 1),
                )
            nc.scalar.activation(
                a1rsb[:], a1row[:], mybir.ActivationFunctionType.Copy
            )
            # transpose to scan layout: A1[o_chunk, (c, n)]
            for c in range(4):
                a1tp = ps1.tile(
                    [PMAX, NB], f32, tag=f"a1tp{c % 2}", name="a1tp"
                )
                nc.tensor.transpose(
                    a1tp[0:MC[c], :],
                    a1rsb[0:NB, OFF[c]:OFF[c] + MC[c]],
                    eyesb[:],
                )
                nc.scalar.activation(
                    A1[0:MC[c], c * NB:(c + 1) * NB],
                    a1tp[0:MC[c], :],
                    mybir.ActivationFunctionType.Copy,
                )

            # ---- u1/20 for layer-1 columns: Up[:, t, j] = A1[:, j] * c20[t] ----
            c20b = c20sb[:].unsqueeze(2).broadcast_to([PMAX, T, 32])
            a1b = A1[:].unsqueeze(1).broadcast_to([PMAX, T, 32])
            nc.vector.scalar_tensor_tensor(
                Up[:, 0:T, 0:32], c20b, 0.0, a1b, Alu.bypass, Alu.mult
            )

            # ---- fused scan: layer-1 at step tau, layer-2 at tau-LAG ----
            # Emit block-b's psp2 pipeline DELAY steps after its last spike
            # step, so the PE matmuls have slack before DVE needs the result.
            MW = M.ap[0][0]          # mega-tile row stride (elements)
            MOFF = M.offset

            def m_ap(off, dims, parts=PMAX):
                return bass.AP(M.tensor, MOFF + off, [[MW, parts]] + dims)

            st_out = m_ap(SW, [[1, 2 * NCOL]])          # [P | R]
            r_in = m_ap(SW + NCOL, [[1, NCOL]])         # R

            DELAY = 13
            blocks = []
            for b in range((T + B - 1) // B):
                tb0, tb1 = b * B, min((b + 1) * B, T)
                blocks.append((tb0, tb1))
            block_at = {tb1 - 1 + DELAY: (tb0, tb1) for tb0, tb1 in blocks}

            p_st = m_ap(SW, [[1, NCOL]])
            for tau in range(TF):
                if tau < T:
                    # full width: 32 layer-1 columns + layer-2 column
                    pp, rr, w0, off = p_st, r_in, NCOL, 0
                else:
                    # tail: layer-1 finished, only column 32 is live
                    pp = m_ap(SW + 32, [[1, 1]])
                    rr = m_ap(SW + NCOL + 32, [[1, 1]])
                    w0, off = 1, 32
                # P = q*P + s_{tau-1}
                nc.vector.scalar_tensor_tensor(
                    pp, pp, q, m_ap(tau * NCOL + off, [[1, w0]]),
                    Alu.mult, Alu.add,
                )
                # R = q*R + P
                nc.vector.scalar_tensor_tensor(
                    rr, rr, q, pp, Alu.mult, Alu.add,
                )
                # s_{tau} = (R + 0.5) <= u/20
                nc.vector.scalar_tensor_tensor(
                    m_ap((tau + 1) * NCOL + off, [[1, w0]]),
                    rr,
                    0.5,
                    Up[:, tau, off:off + w0],
                    Alu.add,
                    Alu.is_le,
                )

                if tau in block_at:
                    tb0, tb1 = block_at[tau]
                    blk = tb1 - tb0
                    # a2[o2, t, n] for t in [tb0, tb1): 4 chunk-matmuls
                    a2ps = ps2.tile([NO2, B, NB], f32, tag="a2ps", name="a2ps")
                    for c in range(4):
                        nc.tensor.matmul(
                            a2ps[:, 0:blk, :],
                            w2sb[:, c, :],
                            m_ap((tb0 + 1) * NCOL + c * NB,
                                 [[NCOL, blk], [1, NB]]),
                            start=(c == 0),
                            stop=(c == 3),
                        )
                    # evac with pre-scale: a2tmp = a2 * e/200  (still [10,(t,n)])
                    nc.scalar.activation(
                        a2tmp[:, 0:blk, :],
                        a2ps[:, 0:blk, :],
                        mybir.ActivationFunctionType.Copy,
                        scale=k2,
                    )
                    # re-stack to [80, t] (row = n*10+o2) via selector matmuls
                    a2r = ps2.tile([80, B], f32, tag="a2r", name="a2r")
                    for n in range(NB):
                        nc.tensor.matmul(
                            a2r[:, 0:blk],
                            selsb[:, n, :],
                            a2tmp[:, 0:blk, n],
                            start=(n == 0),
                            stop=(n == NB - 1),
                        )
                    nc.scalar.activation(
                        a2s[:, tb0 + 1:tb1 + 1],
                        a2r[:, 0:blk],
                        mybir.ActivationFunctionType.Copy,
                    )
                    # W1[t] = (a2s[t-1] + W1[t-1]) * p   (hardware scan)
                    nc.vector.tensor_tensor_scan(
                        W1[:, tb0 + 1:tb1 + 1],
                        a2s[:, tb0:tb1],
                        pcsb[:, 0:blk],
                        W1[:, tb0:tb0 + 1],
                        Alu.add,
                        Alu.mult,
                    )
                    # W2[t] = (W1[t-1] + W2[t-1]) * p
                    nc.vector.tensor_tensor_scan(
                        W2[:, tb0 + 1:tb1 + 1],
                        W1[:, tb0:tb1],
                        pcsb[:, 0:blk],
                        W2[:, tb0:tb0 + 1],
                        Alu.add,
                        Alu.mult,
                    )
                    # u2/20 = W1 + W2 -> layer-2 column of Up, lagged by LAG
                    nc.gpsimd.tensor_tensor(
                        Up[0:80, tb0 + LAG:tb1 + LAG, 32],
                        W1[:, tb0 + 1:tb1 + 1],
                        W2[:, tb0 + 1:tb1 + 1],
                        Alu.add,
                    )

            # ---- output: layer-2 spikes, fused steps LAG..LAG+T ----
            nc.vector.tensor_copy(
                ostage[:],
                m_ap((LAG + 1) * NCOL + 32, [[NCOL, T]], parts=80),
            )
            nc.sync.dma_start(out_d[:], ostage[:])

    nc.compile()
    return nc


def _host_inputs(input, w1, w2):
    f32 = np.float32
    q, p, k2 = _consts()
    flat = np.ascontiguousarray(input.reshape(64, -1).astype(f32))
    flatT = np.ascontiguousarray(flat.T)                      # (3072, 64)
    w1T = np.ascontiguousarray(w1.astype(f32).T)              # (3072, 410)
    w2p = np.zeros((PMAX, 4, NO2), f32)
    for c in range(4):
        w2p[0:MC[c], c, :] = w2.astype(f32)[:, OFF[c]:OFF[c] + MC[c]].T
    t = np.arange(T, dtype=np.float64)
    srm = (t / 10.0) * np.exp(1.0 - t / 10.0)
    c20 = (np.cumsum(srm) / 20.0).astype(f32)
    c20rep = np.broadcast_to(c20, (PMAX, T)).copy()
    pconst = np.full((80, B), p, f32)
    sel = np.zeros((NO2, NB, 80), f32)
    for n in range(NB):
        for o2 in range(NO2):
            sel[o2, n, n * NO2 + o2] = 1.0
    eye8 = np.eye(NB, dtype=f32)
    return flatT, w1T, w2p, c20rep, pconst, sel, eye8


def kernel(input, w1, w2):
    from concourse.bass_utils import run_bass_kernel_spmd

    if "nc" not in _CACHE:
        _CACHE["nc"] = build()
    nc = _CACHE["nc"]

    flatT, w1T, w2p, c20rep, pconst, sel, eye8 = _host_inputs(input, w1, w2)
    in_maps = []
    for core in range(8):
        in_maps.append({
            "flatT": np.ascontiguousarray(flatT[:, core * NB:(core + 1) * NB]),
            "w1T": w1T,
            "w2p": w2p,
            "c20rep": c20rep,
            "pconst": pconst,
            "sel": sel,
            "eye8": eye8,
        })
    res = run_bass_kernel_spmd(nc, in_maps, core_ids=list(range(8)))
    full = np.zeros((64, NO2, T), np.float32)
    for core in range(8):
        full[core * NB:(core + 1) * NB] = (
            res.results[core]["out"].reshape(NB, NO2, T)
        )
    return full

